# revision 5
# baseline (speedup 1.0000x reference)
"""GNN message-passing encoder (3 layers) on 8 Trainium2 NeuronCores.

Sharding: nodes are range-partitioned across the 8 cores (graph/data
parallel). Edges live on the core that owns their dst node, sorted by dst
and padded so every 128-edge tile targets a single 128-node block. Each
layer: per-node projection tables are computed locally (dst table stays
local; src table is AllGathered so any core can gather rows for its
edges' sources), then the edge phase assembles z = Td[dst] + Ts[src] +
ea@We via PE matmuls into PSUM (one-hot expand + K=17 edge projection +
identity-add of the gathered rows), applies sigmoid/softplus via a
Tanh/Silu-only approximation (single ACT table), and scatter-adds the
messages with one-hot matmuls straight into transposed aggregates, which
become the next layer's lhsT without any on-chip transposes.

Precision: bf16 data with f32 PSUM accumulation; weights that multiply
large activations (Wd/Wsrc/Wu) are split hi+lo bf16 so they act as f32.
"""

import sys

sys.path.insert(0, "/opt/trn_rl_repo")

import os
import numpy as np
import ml_dtypes

DBG = os.environ.get("KERNEL_DEBUG_MODE", "full")
NCORES = 8
CHUNK = 2048
A_SP, G_SP = 0.692204, 0.420798  # softplus(x) ~ silu(x) + A*(1 - tanh(G*x)^2)
SENT = 16000

_CACHE = {}


def _bf(x):
    return np.ascontiguousarray(np.asarray(x, np.float32).astype(ml_dtypes.bfloat16))


def _hilo(x):
    x = np.asarray(x, np.float32)
    hi = x.astype(ml_dtypes.bfloat16)
    lo = (x - hi.astype(np.float32)).astype(ml_dtypes.bfloat16)
    return np.ascontiguousarray(hi), np.ascontiguousarray(lo)


def _prep(inputs):
    x = np.asarray(inputs["x"], np.float32)
    ei = np.asarray(inputs["edge_index"])
    ea = np.asarray(inputs["edge_attr"], np.float32)
    N, IN = x.shape
    E, ED = ea.shape
    src_g = ei[0].astype(np.int64)
    dst_g = ei[1].astype(np.int64)
    NPC = N // NCORES
    NBLK = (NPC + 127) // 128

    douts = [inputs[f"Wu{l}"].shape[1] for l in range(3)]
    dins = [inputs[f"Wu{l}"].shape[0] for l in range(3)]

    # ---- edge partitioning: sort by dst, group by (core, block) ----
    order = np.argsort(dst_g, kind="stable")
    ds = dst_g[order]
    key = (ds // NPC) * NBLK + (ds % NPC) // 128
    counts = np.bincount(key, minlength=NCORES * NBLK).reshape(NCORES, NBLK)
    seg_end = np.cumsum(counts.reshape(-1)).reshape(NCORES, NBLK)
    seg_start = seg_end - counts

    T_b = np.maximum(1, -(-counts.max(axis=0) // 128))  # per-block tiles
    E_pc0 = 128 * int(T_b.sum())
    E_PC = -(-E_pc0 // CHUNK) * CHUNK
    T_b[-1] += (E_PC - E_pc0) // 128
    T = E_PC // 128
    blk_of_tile = np.repeat(np.arange(NBLK), T_b)
    tile_off = np.concatenate([[0], np.cumsum(T_b)])  # block -> first tile

    per_core = []
    e_pos = np.arange(E_PC)
    p_of = e_pos % 128
    t_of = e_pos // 128
    for k in range(NCORES):
        src_arr = np.zeros(E_PC, np.int64)
        dstl_arr = np.full(E_PC, SENT, np.int64)
        ea_arr = np.zeros((E_PC, ED), np.float32)
        for b in range(NBLK):
            seg = order[seg_start[k, b] : seg_end[k, b]]
            off = int(tile_off[b]) * 128
            src_arr[off : off + len(seg)] = src_g[seg]
            dstl_arr[off : off + len(seg)] = dst_g[seg] - k * NPC
            ea_arr[off : off + len(seg)] = ea[seg]
        n_loc = dstl_arr - blk_of_tile[t_of] * 128
        valid = (n_loc >= 0) & (n_loc < 128)
        s_sc = np.zeros((128, T * 128), np.float32)
        s_ex = np.zeros((128, T * 128), np.float32)
        s_sc[p_of[valid], t_of[valid] * 128 + n_loc[valid]] = 0.5
        s_ex[n_loc[valid], t_of[valid] * 128 + p_of[valid]] = 1.0
        ea_t = np.ones((ED + 1, E_PC), np.float32)
        ea_t[:ED] = ea_arr.T
        idx = np.ascontiguousarray(np.tile(src_arr.astype(np.int16).reshape(-1, 16).T, (8, 1)))
        x0 = np.zeros((NBLK * 128, IN), np.float32)
        x0[:NPC] = x[k * NPC : (k + 1) * NPC]
        x0t_hi, x0t_lo = _hilo(x0.T)
        per_core.append(
            dict(
                s_sc=_bf(s_sc),
                s_ex=_bf(s_ex),
                ea_t=_bf(ea_t),
                idx=idx,
                x0t_hi=x0t_hi,
                x0t_lo=x0t_lo,
            )
        )

    # ---- weights (shared across cores) ----
    shared = {}
    for l in range(3):
        din, dout = dins[l], douts[l]
        Wf, Ws = np.asarray(inputs[f"Wf{l}"], np.float32), np.asarray(
            inputs[f"Ws{l}"], np.float32
        )
        bfv, bsv = np.asarray(inputs[f"bf{l}"], np.float32), np.asarray(
            inputs[f"bs{l}"], np.float32
        )
        Wu = np.asarray(inputs[f"Wu{l}"], np.float32)
        bu = np.asarray(inputs[f"bu{l}"], np.float32)
        Wd = np.concatenate([Wf[:din], Ws[:din]], 1)  # [din, 2dout]
        Wsr = np.concatenate([Wf[din : 2 * din], Ws[din : 2 * din]], 1)
        Wtab = np.concatenate([Wd, Wsr], 1)  # [din, 4dout]
        KC = min(128, din)
        NK = din // KC
        w_hi, w_lo = _hilo(Wtab)
        # [KC, NK, 4dout]
        shared[f"wtab_hi_{l}"] = np.ascontiguousarray(
            w_hi.reshape(NK, KC, 4 * dout).transpose(1, 0, 2)
        )
        shared[f"wtab_lo_{l}"] = np.ascontiguousarray(
            w_lo.reshape(NK, KC, 4 * dout).transpose(1, 0, 2)
        )
        we = np.concatenate(
            [
                np.concatenate([Wf[2 * din :], Ws[2 * din :]], 1),
                np.concatenate([bfv, bsv])[None],
            ],
            0,
        )  # [ED+1, 2dout]
        shared[f"we_{l}"] = _bf(we)
        wu_hi, wu_lo = _hilo(Wu)
        shared[f"wu_hi_{l}"] = np.ascontiguousarray(
            wu_hi.reshape(NK, KC, dout).transpose(1, 0, 2)
        )
        shared[f"wu_lo_{l}"] = np.ascontiguousarray(
            wu_lo.reshape(NK, KC, dout).transpose(1, 0, 2)
        )
        shared[f"bu_{l}"] = _bf(bu[None])
    shared["ones_r"] = _bf(np.ones((1, NBLK * 128), np.float32))
    shared["ident"] = _bf(np.eye(128, dtype=np.float32))

    cfg = dict(
        N=N,
        E=E,
        IN=IN,
        ED=ED,
        NPC=NPC,
        NBLK=NBLK,
        T=T,
        E_PC=E_PC,
        dins=dins,
        douts=douts,
        blk_of_tile=[int(b) for b in blk_of_tile],
    )
    return cfg, per_core, shared


def _build_program(cfg):
    import concourse.bacc as bacc
    import concourse.mybir as mybir
    import concourse.tile as tile

    bf16 = mybir.dt.bfloat16
    f32 = mybir.dt.float32
    AF = mybir.ActivationFunctionType

    N, ED, NPC, NBLK, T, E_PC = (
        cfg["N"],
        cfg["ED"],
        cfg["NPC"],
        cfg["NBLK"],
        cfg["T"],
        cfg["E_PC"],
    )
    dins, douts = cfg["dins"], cfg["douts"]
    blk_of = cfg["blk_of_tile"]
    IN = cfg["IN"]
    NCH = E_PC // CHUNK

    nc = bacc.Bacc("TRN2", target_bir_lowering=False, debug=False, num_devices=NCORES)

    # ---- dram tensors ----
    d_s_sc = nc.dram_tensor("s_sc", [128, T * 128], bf16, kind="ExternalInput")
    d_s_ex = nc.dram_tensor("s_ex", [128, T * 128], bf16, kind="ExternalInput")
    d_ea = nc.dram_tensor("ea_t", [ED + 1, E_PC], bf16, kind="ExternalInput")
    d_idx = nc.dram_tensor("idx", [128, E_PC // 16], mybir.dt.int16, kind="ExternalInput")
    d_x0hi = nc.dram_tensor("x0t_hi", [IN, NBLK * 128], bf16, kind="ExternalInput")
    d_x0lo = nc.dram_tensor("x0t_lo", [IN, NBLK * 128], bf16, kind="ExternalInput")
    d_w = {}
    for l in range(3):
        din, dout = dins[l], douts[l]
        KC = min(128, din)
        NK = din // KC
        d_w[f"wtab_hi_{l}"] = nc.dram_tensor(
            f"wtab_hi_{l}", [KC, NK, 4 * dout], bf16, kind="ExternalInput"
        )
        d_w[f"wtab_lo_{l}"] = nc.dram_tensor(
            f"wtab_lo_{l}", [KC, NK, 4 * dout], bf16, kind="ExternalInput"
        )
        d_w[f"we_{l}"] = nc.dram_tensor(
            f"we_{l}", [ED + 1, 2 * dout], bf16, kind="ExternalInput"
        )
        d_w[f"wu_hi_{l}"] = nc.dram_tensor(
            f"wu_hi_{l}", [KC, NK, dout], bf16, kind="ExternalInput"
        )
        d_w[f"wu_lo_{l}"] = nc.dram_tensor(
            f"wu_lo_{l}", [KC, NK, dout], bf16, kind="ExternalInput"
        )
        d_w[f"bu_{l}"] = nc.dram_tensor(f"bu_{l}", [1, dout], bf16, kind="ExternalInput")
    d_ones = nc.dram_tensor("ones_r", [1, NBLK * 128], bf16, kind="ExternalInput")
    d_id = nc.dram_tensor("ident", [128, 128], bf16, kind="ExternalInput")
    d_y = nc.dram_tensor("y", [128, NBLK * 128], f32, kind="ExternalOutput")
    d_tsin = [
        nc.dram_tensor(f"ts_in_{l}", [NPC, 2 * douts[l]], bf16) for l in range(3)
    ]
    d_tsfull = [
        nc.dram_tensor(f"ts_full_{l}", [N, 2 * douts[l]], bf16, addr_space="Shared")
        for l in range(3)
    ]

    with tile.TileContext(nc) as tc:
        with (
            tc.tile_pool(name="const", bufs=1) as cpool,
            tc.tile_pool(name="htab", bufs=1) as hpool,
            tc.tile_pool(name="spool", bufs=6) as spool,
            tc.tile_pool(name="gpool", bufs=2) as gpool,
            tc.tile_pool(name="apool", bufs=6) as apool,
            tc.tile_pool(name="stage", bufs=3) as stpool,
            tc.tile_pool(name="epsum", bufs=4, space="PSUM") as epsum,
            tc.tile_pool(name="agg", bufs=4, space="PSUM") as apsum,
        ):
            # ---- load constants ----
            t_ea = cpool.tile([ED + 1, E_PC], bf16, tag="ea")
            nc.sync.dma_start(out=t_ea[:], in_=d_ea[:])
            t_idx = cpool.tile([128, E_PC // 16], mybir.dt.int16, tag="idx")
            nc.sync.dma_start(out=t_idx[:], in_=d_idx[:])
            t_id = cpool.tile([128, 128], bf16, tag="id")
            nc.sync.dma_start(out=t_id[:], in_=d_id[:])
            t_ones = cpool.tile([1, NBLK * 128], bf16, tag="ones")
            nc.sync.dma_start(out=t_ones[:], in_=d_ones[:])
            t_w = {}
            for name, dt_ in d_w.items():
                t_w[name] = cpool.tile(list(dt_.shape), bf16, tag=name, name=f"t_{name}")
                nc.sync.dma_start(out=t_w[name][:], in_=dt_[:])
            t_x0hi = hpool.tile([IN, 1, NBLK * 128], bf16, tag="x0hi")
            nc.sync.dma_start(out=t_x0hi[:, 0, :], in_=d_x0hi[:])
            t_x0lo = hpool.tile([IN, 1, NBLK * 128], bf16, tag="x0lo")
            nc.sync.dma_start(out=t_x0lo[:, 0, :], in_=d_x0lo[:])

            hT_hi, hT_lo = t_x0hi, t_x0lo
            for l in range(3):
                din, dout = dins[l], douts[l]
                KC = min(128, din)
                NK = din // KC
                w_hi, w_lo = t_w[f"wtab_hi_{l}"], t_w[f"wtab_lo_{l}"]
                combos = [(hT_hi, w_hi), (hT_hi, w_lo), (hT_lo, w_hi)]

                # ---- phase A: projection tables ----
                t_td = hpool.tile([128, NBLK, 2 * dout], bf16, tag=f"td_{l}")
                for b in range(NBLK):
                    p_td = epsum.tile([128, 2 * dout], f32, tag="eps")
                    p_ts = epsum.tile([128, 2 * dout], f32, tag="eps")
                    ncall = len(combos) * NK
                    i = 0
                    for hh, ww in combos:
                        for kx in range(NK):
                            lh = hh[:, kx, b * 128 : (b + 1) * 128]
                            nc.tensor.matmul(
                                p_td[:],
                                lh,
                                ww[:, kx, 0 : 2 * dout],
                                start=(i == 0),
                                stop=(i == ncall - 1),
                            )
                            nc.tensor.matmul(
                                p_ts[:],
                                lh,
                                ww[:, kx, 2 * dout : 4 * dout],
                                start=(i == 0),
                                stop=(i == ncall - 1),
                            )
                            i += 1
                    nc.vector.tensor_copy(t_td[:, b, :], p_td[:])
                    t_st = stpool.tile([128, 2 * dout], bf16, tag="ts_stage")
                    nc.vector.tensor_copy(t_st[:], p_ts[:])
                    rows = min(128, NPC - b * 128)
                    nc.sync.dma_start(
                        out=d_tsin[l][b * 128 : b * 128 + rows, :], in_=t_st[:rows, :]
                    )
                if DBG == "nocoll":
                    nc.sync.dma_start(out=d_tsfull[l][0:NPC, :], in_=d_tsin[l][:])
                else:
                    nc.gpsimd.collective_compute(
                        "AllGather",
                        mybir.AluOpType.bypass,
                        replica_groups=[list(range(NCORES))],
                        ins=[d_tsin[l][:]],
                        outs=[d_tsfull[l][:]],
                    )

                # ---- phase B: edge phase ----
                agg = {}  # (b, mi) -> psum tile
                started = set()
                MI = dout // 128
                last_tile_of_blk = {}
                for t in range(T):
                    last_tile_of_blk[blk_of[t]] = t
                for c in range(NCH):
                    t_g = gpool.tile([128, CHUNK // 128, 2 * dout], bf16, tag="gath")
                    if DBG == "nogather":
                        nc.gpsimd.memset(t_g[:], 0.0)
                    else:
                        nc.gpsimd.dma_gather(
                            out_ap=t_g[:],
                            in_ap=d_tsfull[l][:],
                            idxs_ap=t_idx[
                                :, c * (CHUNK // 16) : (c + 1) * (CHUNK // 16)
                            ],
                            num_idxs=CHUNK,
                            num_idxs_reg=CHUNK,
                            elem_size=2 * dout,
                            single_packet=False,
                        )
                    t_ssc = spool.tile([128, CHUNK // 128, 128], bf16, tag="ssc")
                    nc.sync.dma_start(
                        out=t_ssc[:], in_=d_s_sc[:, c * CHUNK : (c + 1) * CHUNK]
                    )
                    t_sex = spool.tile([128, CHUNK // 128, 128], bf16, tag="sex")
                    nc.sync.dma_start(
                        out=t_sex[:], in_=d_s_ex[:, c * CHUNK : (c + 1) * CHUNK]
                    )
                    for i in range(CHUNK // 128):
                        t = c * (CHUNK // 128) + i
                        b = blk_of[t]
                        pe = epsum.tile([128, 2 * dout], f32, tag="eps")
                        nc.tensor.matmul(
                            pe[:], t_sex[:, i, :], t_td[:, b, :], start=True, stop=False
                        )
                        nc.tensor.matmul(
                            pe[:],
                            t_ea[:, t * 128 : (t + 1) * 128],
                            t_w[f"we_{l}"][:],
                            start=False,
                            stop=False,
                        )
                        nc.tensor.matmul(
                            pe[:], t_id[:], t_g[:, i, :], start=False, stop=True
                        )
                        t_u = apool.tile([128, dout], bf16, tag="u")
                        nc.scalar.activation(t_u[:], pe[:, :dout], AF.Tanh, scale=0.5)
                        t_v = apool.tile([128, dout], bf16, tag="v")
                        nc.scalar.activation(t_v[:], pe[:, dout:], AF.Silu)
                        t_t = apool.tile([128, dout], bf16, tag="t")
                        nc.scalar.activation(t_t[:], pe[:, dout:], AF.Tanh, scale=G_SP)
                        t_sq = apool.tile([128, dout], bf16, tag="sq")
                        nc.vector.tensor_mul(t_sq[:], t_t[:], t_t[:])
                        t_wv = apool.tile([128, dout], bf16, tag="w")
                        if DBG == "nocustom":
                            nc.vector.tensor_scalar(
                                out=t_wv[:], in0=t_sq[:], scalar1=-A_SP, scalar2=A_SP,
                                op0=mybir.AluOpType.mult, op1=mybir.AluOpType.add)
                            nc.vector.tensor_add(out=t_wv[:], in0=t_wv[:], in1=t_v[:])
                        else:
                            nc.vector.affine_then_add(t_wv[:], t_sq[:], t_v[:], -A_SP, A_SP)
                        t_p = apool.tile([128, dout], bf16, tag="p")
                        nc.vector.scalar_tensor_tensor(
                            t_p[:],
                            t_u[:],
                            1.0,
                            t_wv[:],
                            mybir.AluOpType.add,
                            mybir.AluOpType.mult,
                        )
                        for mi in range(MI):
                            if (b, mi) not in agg:
                                agg[b, mi] = apsum.tile([128, 128], f32, tag="agg", name=f"agg_{l}_{b}_{mi}")
                            nc.tensor.matmul(
                                agg[b, mi][:],
                                t_p[:, mi * 128 : (mi + 1) * 128],
                                t_ssc[:, i, :],
                                start=(b, mi) not in started,
                                stop=False,
                            )
                            started.add((b, mi))
                        # ---- block close ----
                        if t == last_tile_of_blk[b]:
                            wu_hi, wu_lo = t_w[f"wu_hi_{l}"], t_w[f"wu_lo_{l}"]
                            ucombos = [
                                (hT_hi, wu_hi),
                                (hT_hi, wu_lo),
                                (hT_lo, wu_hi),
                            ]
                            for mi in range(MI):
                                for hh, ww in ucombos:
                                    for kx in range(NK):
                                        nc.tensor.matmul(
                                            agg[b, mi][:],
                                            ww[:, kx, mi * 128 : (mi + 1) * 128],
                                            hh[:, kx, b * 128 : (b + 1) * 128],
                                            start=False,
                                            stop=False,
                                        )
                                nc.tensor.matmul(
                                    agg[b, mi][:],
                                    t_w[f"bu_{l}"][:, mi * 128 : (mi + 1) * 128],
                                    t_ones[:, b * 128 : (b + 1) * 128],
                                    start=False,
                                    stop=True,
                                )
                            if l < 2:
                                if b == 0:
                                    hT_hi_n = hpool.tile(
                                        [128, MI, NBLK * 128], bf16, tag=f"h{l}hi"
                                    )
                                    hT_lo_n = hpool.tile(
                                        [128, MI, NBLK * 128], bf16, tag=f"h{l}lo"
                                    )
                                for mi in range(MI):
                                    nc.vector.tensor_copy(
                                        hT_hi_n[:, mi, b * 128 : (b + 1) * 128],
                                        agg[b, mi][:],
                                    )
                                    nc.vector.tensor_tensor(
                                        out=hT_lo_n[:, mi, b * 128 : (b + 1) * 128],
                                        in0=agg[b, mi][:],
                                        in1=hT_hi_n[:, mi, b * 128 : (b + 1) * 128],
                                        op=mybir.AluOpType.subtract,
                                    )
                            else:
                                t_y = stpool.tile([128, 128], f32, tag="ystage")
                                nc.vector.tensor_copy(t_y[:], agg[b, 0][:])
                                nc.sync.dma_start(
                                    out=d_y[:, b * 128 : (b + 1) * 128], in_=t_y[:]
                                )
                if l < 2:
                    hT_hi, hT_lo = hT_hi_n, hT_lo_n

    nc.compile()
    return nc


def kernel(**inputs):
    from concourse.bass_utils import run_bass_kernel_spmd

    cfg, per_core, shared = _prep(inputs)
    key = (cfg["N"], cfg["E"], cfg["E_PC"])
    if key not in _CACHE:
        _CACHE[key] = _build_program(cfg)
    nc = _CACHE[key]

    in_maps = [{**pc, **shared} for pc in per_core]
    res = run_bass_kernel_spmd(nc, in_maps, list(range(NCORES)))

    NPC = cfg["NPC"]
    out = np.concatenate(
        [res.results[k]["y"][:, :NPC].T for k in range(NCORES)], axis=0
    )
    G = int(np.asarray(inputs["num_graphs"]))
    return out.reshape(G, -1, cfg["douts"][2]).astype(np.float32)
